# revision 1
# baseline (speedup 1.0000x reference)
"""Multi-head attention (RoPE, causal) Bass kernel for 8 TRN2 NeuronCores.

Sharding: 2-way batch x 4-way heads (4 heads per core).
Per-core inputs (DRAM, float32r unless noted):
  xT   [1024, 2048]  x[b].T
  wq/wk/wv [1024, 256]  per-head-group columns
  wo   [256, 1024]   per-head-group rows
  cdup/sdup [128, 2048] float32 RoPE tables (duplicated per stacked head pair)
  p64  [128, 128]    rotate-half partition permutation
  tri  [128, 128]    causal mask for diagonal blocks (j <= i)
  onesc [128, 65]    ones
Output: out [2048, 1024] partial (summed over the 4 head-group cores on host).

Layout notes:
  QT/KT stacked [128, S]: partitions 0-63 head even, 64-127 head odd, f32r.
  Scores computed transposed: ST[j-chunk 128, i 512] per head; softmax
  denominator comes free from an appended ones-column on V (PSUM row 64).
  Normalization: DVE reciprocal of the l-row -> gpsimd partition_broadcast
  -> DVE multiply; deferred by one i-slice so the PE never waits on it.
"""
import numpy as np
from contextlib import ExitStack

import concourse.bass as bass
import concourse.tile as tile
from concourse import bacc, mybir
from concourse.bass_utils import run_bass_kernel_spmd

D_IN = 1024
D_OUT = 1024
HD = 64                   # head dim
S = 2048                  # sequence length
B = 2
THETA = 10000.0
NCORES = 8
IS = 512                  # i-slice width
NIS = S // IS             # 4 i-slices
NJC = S // 128            # 16 j-chunks

F32 = mybir.dt.float32
F32R = mybir.dt.float32r


def build_kernel():
    nc = bacc.Bacc("TRN2", target_bir_lowering=False, debug=False)

    # host pre-shuffled so every DMA is contiguous per partition:
    # xtr[it, p, c, i] = x[b]^T[128c+p, 512it+i]; w*r[p, c, n] = W[128c+p, n]
    xT = nc.dram_tensor("xT", [NIS, 128, 8, IS], F32R, kind="ExternalInput").ap()
    wq = nc.dram_tensor("wq", [128, 8, 256], F32R, kind="ExternalInput").ap()
    wk = nc.dram_tensor("wk", [128, 8, 256], F32R, kind="ExternalInput").ap()
    wv = nc.dram_tensor("wv", [128, 8, 256], F32R, kind="ExternalInput").ap()
    wo = nc.dram_tensor("wo", [128, 2, 1024], F32R, kind="ExternalInput").ap()
    cdup = nc.dram_tensor("cdup", [128, S], F32, kind="ExternalInput").ap()
    sdup = nc.dram_tensor("sdup", [128, S], F32, kind="ExternalInput").ap()
    p64 = nc.dram_tensor("p64", [128, 128], F32R, kind="ExternalInput").ap()
    tri = nc.dram_tensor("tri", [128, 128], F32R, kind="ExternalInput").ap()
    onesc = nc.dram_tensor("onesc", [128, 65], F32R, kind="ExternalInput").ap()
    out = nc.dram_tensor("out", [S, D_OUT], F32, kind="ExternalOutput").ap()

    with tile.TileContext(nc) as tc, ExitStack() as ctx:
        singles = ctx.enter_context(tc.tile_pool(name="singles", bufs=1))
        xpool = ctx.enter_context(tc.tile_pool(name="xpool", bufs=2))
        rope_tmp = ctx.enter_context(tc.tile_pool(name="rope_tmp", bufs=3))
        expp = ctx.enter_context(tc.tile_pool(name="expp", bufs=3))
        bcp = ctx.enter_context(tc.tile_pool(name="bcp", bufs=2))
        ctxp = ctx.enter_context(tc.tile_pool(name="ctxp", bufs=2))
        outp = ctx.enter_context(tc.tile_pool(name="outp", bufs=3))
        # PSUM: ps_a 4x1 bank + ps_b 2x2 banks = 8 banks
        ps_a = ctx.enter_context(tc.tile_pool(name="ps_a", bufs=4, space="PSUM"))
        ps_b = ctx.enter_context(tc.tile_pool(name="ps_b", bufs=2, space="PSUM"))

        # ---- weights / tables, ordered by first use ----
        # (wq/wk/xt0 gate the very first matmuls; wv/tri later; wo last)
        def w_dma(name, ap, split=1):
            # split per chunk-group so the first matmuls wait on less data
            t = singles.tile([128, 8, 256], F32R, tag=name, name=name)
            step = 8 // split
            for s in range(split):
                nc.sync.dma_start(out=t[:, s * step:(s + 1) * step, :],
                                  in_=ap[:, s * step:(s + 1) * step, :])
            return t

        xts = {}

        def xt_dma(it, split=1):
            t = xpool.tile([128, 8, IS], F32R, tag="xt", name=f"xt{it}")
            for s in range(0, 8, 8 // split):
                nc.sync.dma_start(out=t[:, s:s + 8 // split, :],
                                  in_=xT[it, :, s:s + 8 // split, :])
            xts[it] = t

        # interleave wq/xt0 chunk DMAs: the first projection chain consumes
        # (wq chunk c, xt0 chunk c) in order
        wq_t = singles.tile([128, 8, 256], F32R, tag="wq", name="wq")
        xt0_t = xpool.tile([128, 8, IS], F32R, tag="xt", name="xt0")
        for c in range(8):
            nc.sync.dma_start(out=wq_t[:, c, :], in_=wq[:, c, :])
            nc.sync.dma_start(out=xt0_t[:, c, :], in_=xT[0, :, c, :])
        xts[0] = xt0_t
        w_sb = {"wq": wq_t, "wk": w_dma("wk", wk, split=4)}
        c_sb = singles.tile([128, S], F32, tag="cdup")
        nc.sync.dma_start(out=c_sb, in_=cdup)
        s_sb = singles.tile([128, S], F32, tag="sdup")
        nc.sync.dma_start(out=s_sb, in_=sdup)
        p64_sb = singles.tile([128, 128], F32R, tag="p64")
        nc.sync.dma_start(out=p64_sb, in_=p64)
        w_sb["wv"] = w_dma("wv", wv)
        onesc_sb = singles.tile([128, 65], F32R, tag="ones")
        nc.sync.dma_start(out=onesc_sb, in_=onesc)
        tri_sb = singles.tile([128, 128], F32R, tag="tri")
        nc.sync.dma_start(out=tri_sb, in_=tri)
        xt_dma(1)
        # deferred: wo is not needed until the first out-projection
        wo_sb = singles.tile([128, 2, 1024], F32R, tag="wo")

        # persistent SBUF state
        qt = [singles.tile([128, S], F32R, tag=f"qt{p}", name=f"qt{p}")
              for p in range(2)]
        kt = [singles.tile([128, S], F32R, tag=f"kt{p}", name=f"kt{p}")
              for p in range(2)]
        v4 = singles.tile([128, NJC, 4, 65], F32R, tag="v4")
        v4_ones = bass.AP(tensor=v4.tensor, offset=64,
                          ap=[[NJC * 4 * 65, 128], [65, NJC * 4]])
        nc.vector.tensor_copy(v4_ones, onesc_sb[:, 0:64])
        # 1/l rows at partition 0: [head, it parity, i]
        rl = singles.tile([128, 4, 2, IS], F32, tag="rl")

        # ================= Phase 1: QKV projections + RoPE =================
        for it in range(NIS):
            xt_t = xts[it]
            for tname, wt, dests in (("q", w_sb["wq"], qt), ("k", w_sb["wk"], kt)):
                for p in range(2):
                    proj = ps_a.tile([128, IS], F32, tag="a", name="proj")
                    for c in range(8):
                        nc.tensor.matmul(proj, wt[:, c, 128 * p:128 * (p + 1)],
                                         xt_t[:, c, :],
                                         start=(c == 0), stop=(c == 7))
                    # RoPE: rot = proj * cos + (P64 @ proj) * sin'
                    raw = rope_tmp.tile([128, IS], F32R, tag="raw")
                    nc.scalar.copy(raw, proj)
                    perm = ps_b.tile([128, 2, IS], F32, tag="b", name="perm")
                    nc.tensor.matmul(perm[:, 0, :], p64_sb, raw,
                                     start=True, stop=True)
                    t1 = rope_tmp.tile([128, IS], F32, tag="t1")
                    nc.vector.tensor_mul(t1, proj, c_sb[:, it * IS:(it + 1) * IS])
                    t2 = rope_tmp.tile([128, IS], F32, tag="t2")
                    nc.vector.tensor_mul(t2, perm[:, 0, :],
                                         s_sb[:, it * IS:(it + 1) * IS])
                    nc.vector.tensor_add(
                        dests[p][:, it * IS:(it + 1) * IS], t1, t2)

            # V projection: [j, 256] tiles, 4 j-subtiles per i-slice
            for half in range(2):
                vps = ps_b.tile([128, 2, 256], F32, tag="b", name="vps")
                for js in range(2):
                    jt = it * 4 + half * 2 + js
                    for c in range(8):
                        nc.tensor.matmul(
                            vps[:, js, :],
                            xt_t[:, c, 128 * (half * 2 + js):128 * (half * 2 + js + 1)],
                            w_sb["wv"][:, c, :],
                            start=(c == 0), stop=(c == 7))
                for js in range(2):
                    jt = it * 4 + half * 2 + js
                    nc.vector.tensor_copy(v4[:, jt, :, 0:64],
                                          vps[:, js, :].rearrange("p (h d) -> p h d", h=4))
            if it == 0:
                nc.sync.dma_start(out=wo_sb, in_=wo)
            if it + 2 < NIS:
                xt_dma(it + 2)

        # ============ Phase 2: attention, norm/out-proj deferred 1 it ======
        def attention_head(pair, half, it):
            h = 2 * pair + half
            hb = 64 * half
            qs = qt[pair][hb:hb + 64, :]
            ks = kt[pair][hb:hb + 64, :]
            njc = 4 * it + 4
            ctx_ps = ps_a.tile([128, IS], F32, tag="a", name=f"ctx{h}")
            for q0 in range(0, njc, 2):
                nq = min(2, njc - q0)
                quad = ps_b.tile([128, 2, IS], F32, tag="b", name="quad")
                exps = expp.tile([128, 2, IS], F32R, tag="e", name="exps")
                for qi in range(nq):
                    jc = q0 + qi
                    c0 = max(0, 128 * (jc - 4 * it))
                    nc.tensor.matmul(
                        quad[:, qi, c0:IS],
                        ks[:, 128 * jc:128 * (jc + 1)],
                        qs[:, it * IS + c0:(it + 1) * IS],
                        start=True, stop=True)
                nc.scalar.activation(
                    exps[:, 0:nq, :], quad[:, 0:nq, :],
                    mybir.ActivationFunctionType.Exp, scale=0.125)
                for qi in range(nq):
                    jc = q0 + qi
                    c0 = max(0, 128 * (jc - 4 * it))
                    if c0 > 0 or jc == 4 * it:
                        # causal mask of the diagonal block
                        nc.vector.tensor_mul(
                            exps[:, qi, c0:c0 + 128],
                            exps[:, qi, c0:c0 + 128], tri_sb)
                    nc.tensor.matmul(
                        ctx_ps[0:65, c0:IS],
                        v4[:, jc, h, :],
                        exps[:, qi, c0:IS],
                        start=(jc == 0), stop=(jc == njc - 1))
            return ctx_ps

        def recip_l(ctx_ps, h, it):
            # 1/l of row 64, written to partition 0 (DVE iterative divide,
            # ~3.4us; runs in the shadow of the next head's attention)
            with nc.allow_low_precision(reason="fp32 width"):
                nc.vector.reciprocal(rl[0:1, h, it % 2, :],
                                     ctx_ps[64:65, :])

        def norm_and_outproj(ctx_tiles, it):
            ctxs_pair = []
            for pair in range(2):
                ctxs = ctxp.tile([128, IS], F32R, tag="c", name="ctxs")
                for half in range(2):
                    bcs = bcp.tile([64, IS], F32, tag="bc", name="bcs")
                    nc.gpsimd.partition_broadcast(
                        bcs, rl[0:1, 2 * pair + half, it % 2, :])
                    nc.vector.tensor_mul(
                        ctxs[64 * half:64 * half + 64, :],
                        ctx_tiles[2 * pair + half][0:64, :], bcs)
                ctxs_pair.append(ctxs)
            for ib in range(4):
                ot = outp.tile([128, 1024], F32, tag="o", name="ot")
                for nt in range(2):
                    ops = ps_a.tile([128, IS], F32, tag="a", name="ops")
                    for pair in range(2):
                        nc.tensor.matmul(
                            ops,
                            ctxs_pair[pair][:, 128 * ib:128 * (ib + 1)],
                            wo_sb[:, pair, nt * IS:(nt + 1) * IS],
                            start=(pair == 0), stop=(pair == 1))
                    if nt == 0:
                        nc.vector.tensor_copy(ot[:, 0:IS], ops)
                    else:
                        nc.scalar.copy(ot[:, IS:1024], ops)
                nc.sync.dma_start(
                    out=out[it * IS + 128 * ib: it * IS + 128 * (ib + 1), :],
                    in_=ot)

        # Issue order: norm(it-1) goes after head0(it)'s matmuls but BEFORE
        # recip(h0, it), so on the DVE FIFO the norm multiplies are not
        # queued behind a fresh 3.4us reciprocal.
        pending = None
        for it in range(NIS):
            ctx_tiles = {}
            ctx_tiles[0] = attention_head(0, 0, it)
            if pending is not None:
                norm_and_outproj(*pending)
            recip_l(ctx_tiles[0], 0, it)
            for h, (pair, half) in enumerate([(0, 1), (1, 0), (1, 1)], start=1):
                ctx_tiles[h] = attention_head(pair, half, it)
                recip_l(ctx_tiles[h], h, it)
            pending = (ctx_tiles, it)
        norm_and_outproj(*pending)

    nc.compile()
    return nc


def _host_tables():
    inv_freq = 1.0 / (THETA ** (np.arange(0, HD, 2, dtype=np.float64) / HD))
    pos = np.arange(S, dtype=np.float64)
    ang = pos[None, :] * inv_freq[:, None]          # [32, S]
    cos32 = np.cos(ang).astype(np.float32)
    sin32 = np.sin(ang).astype(np.float32)
    cdup = np.concatenate([cos32, cos32, cos32, cos32], axis=0)  # [128, S]
    s_signed = np.concatenate([-sin32, sin32, -sin32, sin32], axis=0)
    p64 = np.zeros((128, 128), dtype=np.float32)
    for m in range(128):
        blk = m - (m % 64)
        d = m % 64
        p64[blk + ((d + 32) % 64), m] = 1.0
    tri = (np.arange(128)[:, None] <= np.arange(128)[None, :]).astype(np.float32)
    return cdup, s_signed, p64, tri


_NC_CACHE = {}


def make_in_maps(x, W_q, W_k, W_v, W_o):
    cdup, sdup, p64, tri = _host_tables()
    ones = np.ones((128, 65), dtype=np.float32)
    def wshuf(w):  # [1024, 256] -> [128, 8, 256]
        return np.ascontiguousarray(w.reshape(8, 128, 256).transpose(1, 0, 2))

    in_maps = []
    for c in range(NCORES):
        b, g = divmod(c, 4)
        cols = slice(256 * g, 256 * (g + 1))
        # xtr[it, p, ch, i] = x[b][512it+i, 128ch+p]
        xtr = np.ascontiguousarray(
            x[b].reshape(NIS, IS, 8, 128).transpose(0, 3, 2, 1))
        in_maps.append({
            "xT": xtr,
            "wq": wshuf(W_q[:, cols]),
            "wk": wshuf(W_k[:, cols]),
            "wv": wshuf(W_v[:, cols]),
            "wo": np.ascontiguousarray(
                W_o[cols, :].reshape(2, 128, 1024).transpose(1, 0, 2)),
            "cdup": cdup, "sdup": sdup, "p64": p64, "tri": tri,
            "onesc": ones,
        })
    return in_maps


def kernel(x, W_q, W_k, W_v, W_o):
    x = np.ascontiguousarray(x, dtype=np.float32)
    W_q = np.ascontiguousarray(W_q, dtype=np.float32)
    W_k = np.ascontiguousarray(W_k, dtype=np.float32)
    W_v = np.ascontiguousarray(W_v, dtype=np.float32)
    W_o = np.ascontiguousarray(W_o, dtype=np.float32)

    if "nc" not in _NC_CACHE:
        _NC_CACHE["nc"] = build_kernel()
    nc = _NC_CACHE["nc"]

    in_maps = make_in_maps(x, W_q, W_k, W_v, W_o)
    res = run_bass_kernel_spmd(nc, in_maps, list(range(NCORES)))
    outs = [res.results[c]["out"] for c in range(NCORES)]
    full = np.empty((B, S, D_OUT), dtype=np.float32)
    for b in range(B):
        full[b] = outs[4 * b] + outs[4 * b + 1] + outs[4 * b + 2] + outs[4 * b + 3]
    return full



# revision 4
# speedup vs baseline: 1.3204x; 1.3204x over previous
"""Multi-head attention (RoPE, causal) Bass kernel for 8 TRN2 NeuronCores.

Sharding: 2-way batch x 4-way heads (4 heads per core).
v2 changes vs baseline:
  - bf16 inputs/weights/tables (DMA halved, FWL weight loads, faster start)
  - causal mask folded into score PSUM via identity-matmul bias add
    (-30k upper triangle) -> no DVE op between exp and ctx matmul
  - softmax 1/l via reciprocal_approx_fast (single DVE pass, ~5x faster)
  - exp calls column-trimmed to the causal region per chunk pair
  - out-projection PSUM->SBUF copies moved off the Scalar engine
    (DVE + gpsimd) so ACT does exp only during attention
  - bf16 partial outputs, summed on host in f32

Layout notes:
  QT/KT stacked [128, S]: partitions 0-63 head even, 64-127 head odd, bf16.
  Scores computed transposed: ST[j-chunk 128, i 512] per head; softmax
  denominator comes free from an appended ones-column on V (PSUM row 64).
"""
import numpy as np
from contextlib import ExitStack

import ml_dtypes
import concourse.bass as bass
import concourse.tile as tile
from concourse import bacc, mybir
from concourse.bass_utils import run_bass_kernel_spmd

D_IN = 1024
D_OUT = 1024
HD = 64                   # head dim
S = 2048                  # sequence length
B = 2
THETA = 10000.0
NCORES = 8
IS = 512                  # i-slice width
NIS = S // IS             # 4 i-slices
NJC = S // 128            # 16 j-chunks
NEG = -30000.0            # causal mask bias (exp(NEG/8) == 0 in f32)

F32 = mybir.dt.float32
BF16 = mybir.dt.bfloat16
BFNP = ml_dtypes.bfloat16


def build_kernel():
    nc = bacc.Bacc("TRN2", target_bir_lowering=False, debug=False)

    # host pre-shuffled so every DMA is contiguous per partition:
    # xtr[it, p, c, i] = x[b]^T[128c+p, 512it+i]; w*r[p, c, n] = W[128c+p, n]
    xT = nc.dram_tensor("xT", [NIS, 128, 8, IS], BF16, kind="ExternalInput").ap()
    wq = nc.dram_tensor("wq", [128, 8, 256], BF16, kind="ExternalInput").ap()
    wk = nc.dram_tensor("wk", [128, 8, 256], BF16, kind="ExternalInput").ap()
    wv = nc.dram_tensor("wv", [128, 8, 256], BF16, kind="ExternalInput").ap()
    wo = nc.dram_tensor("wo", [128, 2, 1024], BF16, kind="ExternalInput").ap()
    cdup = nc.dram_tensor("cdup", [128, S], BF16, kind="ExternalInput").ap()
    sdup = nc.dram_tensor("sdup", [128, S], BF16, kind="ExternalInput").ap()
    p64 = nc.dram_tensor("p64", [128, 128], BF16, kind="ExternalInput").ap()
    ident = nc.dram_tensor("ident", [128, 128], BF16, kind="ExternalInput").ap()
    tneg = nc.dram_tensor("tneg", [128, 128], BF16, kind="ExternalInput").ap()
    onesc = nc.dram_tensor("onesc", [128, 65], BF16, kind="ExternalInput").ap()
    out = nc.dram_tensor("out", [S, D_OUT], BF16, kind="ExternalOutput").ap()

    with tile.TileContext(nc) as tc, ExitStack() as ctx:
        singles = ctx.enter_context(tc.tile_pool(name="singles", bufs=1))
        xpool = ctx.enter_context(tc.tile_pool(name="xpool", bufs=2))
        rope_tmp = ctx.enter_context(tc.tile_pool(name="rope_tmp", bufs=3))
        expp = ctx.enter_context(tc.tile_pool(name="expp", bufs=3))
        bcp = ctx.enter_context(tc.tile_pool(name="bcp", bufs=2))
        ctxp = ctx.enter_context(tc.tile_pool(name="ctxp", bufs=2))
        outp = ctx.enter_context(tc.tile_pool(name="outp", bufs=3))
        # PSUM: ps_a 4x1 bank + ps_b 2x2 banks = 8 banks
        ps_a = ctx.enter_context(tc.tile_pool(name="ps_a", bufs=4, space="PSUM"))
        ps_b = ctx.enter_context(tc.tile_pool(name="ps_b", bufs=2, space="PSUM"))

        # ---- weights / tables, ordered by first use ----
        def w_dma(name, ap, split=1):
            t = singles.tile([128, 8, 256], BF16, tag=name, name=name)
            step = 8 // split
            for s in range(split):
                nc.sync.dma_start(out=t[:, s * step:(s + 1) * step, :],
                                  in_=ap[:, s * step:(s + 1) * step, :])
            return t

        xts = {}

        def xt_dma(it, split=1):
            t = xpool.tile([128, 8, IS], BF16, tag="xt", name=f"xt{it}")
            for s in range(0, 8, 8 // split):
                nc.sync.dma_start(out=t[:, s:s + 8 // split, :],
                                  in_=xT[it, :, s:s + 8 // split, :])
            xts[it] = t

        # interleave wq/xt0 chunk DMAs: the first projection chain consumes
        # (wq chunk c, xt0 chunk c) in order
        wq_t = singles.tile([128, 8, 256], BF16, tag="wq", name="wq")
        xt0_t = xpool.tile([128, 8, IS], BF16, tag="xt", name="xt0")
        for c in range(8):
            nc.sync.dma_start(out=wq_t[:, c, :], in_=wq[:, c, :])
            nc.sync.dma_start(out=xt0_t[:, c, :], in_=xT[0, :, c, :])
        xts[0] = xt0_t
        w_sb = {"wq": wq_t, "wk": w_dma("wk", wk, split=4)}
        c_sb = singles.tile([128, S], BF16, tag="cdup")
        nc.sync.dma_start(out=c_sb, in_=cdup)
        s_sb = singles.tile([128, S], BF16, tag="sdup")
        nc.sync.dma_start(out=s_sb, in_=sdup)
        p64_sb = singles.tile([128, 128], BF16, tag="p64")
        nc.sync.dma_start(out=p64_sb, in_=p64)
        w_sb["wv"] = w_dma("wv", wv)
        onesc_sb = singles.tile([128, 65], BF16, tag="ones")
        nc.sync.dma_start(out=onesc_sb, in_=onesc)
        ident_sb = singles.tile([128, 128], BF16, tag="ident")
        nc.sync.dma_start(out=ident_sb, in_=ident)
        tneg_sb = singles.tile([128, 128], BF16, tag="tneg")
        nc.sync.dma_start(out=tneg_sb, in_=tneg)
        xt_dma(1)
        # deferred: wo is not needed until the first out-projection
        wo_sb = singles.tile([128, 2, 1024], BF16, tag="wo")

        # persistent SBUF state
        qt = [singles.tile([128, S], BF16, tag=f"qt{p}", name=f"qt{p}")
              for p in range(2)]
        kt = [singles.tile([128, S], BF16, tag=f"kt{p}", name=f"kt{p}")
              for p in range(2)]
        v4 = singles.tile([128, NJC, 4, 65], BF16, tag="v4")
        v4_ones = bass.AP(tensor=v4.tensor, offset=64,
                          ap=[[NJC * 4 * 65, 128], [65, NJC * 4]])
        nc.vector.tensor_copy(v4_ones, onesc_sb[:, 0:64])
        # 1/l rows at partition 0: [head, it parity, i]
        rl = singles.tile([128, 4, 2, IS], F32, tag="rl")

        # ================= Phase 1: QKV projections + RoPE =================
        for it in range(NIS):
            xt_t = xts[it]
            for tname, wt, dests in (("q", w_sb["wq"], qt), ("k", w_sb["wk"], kt)):
                for p in range(2):
                    proj = ps_a.tile([128, IS], F32, tag="a", name="proj")
                    for c in range(8):
                        nc.tensor.matmul(proj, wt[:, c, 128 * p:128 * (p + 1)],
                                         xt_t[:, c, :],
                                         start=(c == 0), stop=(c == 7))
                    # RoPE: rot = proj * cos + (P64 @ proj) * sin'
                    raw = rope_tmp.tile([128, IS], BF16, tag="raw")
                    nc.scalar.copy(raw, proj)
                    perm = ps_b.tile([128, 2, IS], F32, tag="b", name="perm")
                    nc.tensor.matmul(perm[:, 0, :], p64_sb, raw,
                                     start=True, stop=True)
                    t1 = rope_tmp.tile([128, IS], BF16, tag="t1")
                    nc.vector.tensor_mul(t1, proj, c_sb[:, it * IS:(it + 1) * IS])
                    t2 = rope_tmp.tile([128, IS], BF16, tag="t2")
                    nc.vector.tensor_mul(t2, perm[:, 0, :],
                                         s_sb[:, it * IS:(it + 1) * IS])
                    nc.vector.tensor_add(
                        dests[p][:, it * IS:(it + 1) * IS], t1, t2)

            # V projection: [j, 256] tiles, 4 j-subtiles per i-slice
            for half in range(2):
                vps = ps_b.tile([128, 2, 256], F32, tag="b", name="vps")
                for js in range(2):
                    jt = it * 4 + half * 2 + js
                    for c in range(8):
                        nc.tensor.matmul(
                            vps[:, js, :],
                            xt_t[:, c, 128 * (half * 2 + js):128 * (half * 2 + js + 1)],
                            w_sb["wv"][:, c, :],
                            start=(c == 0), stop=(c == 7))
                for js in range(2):
                    jt = it * 4 + half * 2 + js
                    nc.vector.tensor_copy(v4[:, jt, :, 0:64],
                                          vps[:, js, :].rearrange("p (h d) -> p h d", h=4))
            if it == 0:
                nc.sync.dma_start(out=wo_sb, in_=wo)
            if it + 2 < NIS:
                xt_dma(it + 2)

        # ============ Phase 2: attention, norm/out-proj deferred 1 it ======
        def attention_head(pair, half, it):
            h = 2 * pair + half
            hb = 64 * half
            qs = qt[pair][hb:hb + 64, :]
            ks = kt[pair][hb:hb + 64, :]
            njc = 4 * it + 4
            ctx_ps = ps_a.tile([128, IS], F32, tag="a", name=f"ctx{h}")
            for q0 in range(0, njc, 2):
                nq = min(2, njc - q0)
                quad = ps_b.tile([128, 2, IS], F32, tag="b", name="quad")
                exps = expp.tile([128, 2, IS], BF16, tag="e", name="exps")
                cmin = max(0, 128 * (q0 - 4 * it))
                for qi in range(nq):
                    jc = q0 + qi
                    c0 = max(0, 128 * (jc - 4 * it))
                    diag = jc >= 4 * it
                    nc.tensor.matmul(
                        quad[:, qi, c0:IS],
                        ks[:, 128 * jc:128 * (jc + 1)],
                        qs[:, it * IS + c0:(it + 1) * IS],
                        start=True, stop=not diag)
                    if diag:
                        # add -30k upper-triangle bias into the diagonal block
                        nc.tensor.matmul(
                            quad[:, qi, c0:c0 + 128],
                            ident_sb, tneg_sb,
                            start=False, stop=True)
                nc.scalar.activation(
                    exps[:, 0:nq, cmin:IS], quad[:, 0:nq, cmin:IS],
                    mybir.ActivationFunctionType.Exp, scale=0.125)
                for qi in range(nq):
                    jc = q0 + qi
                    c0 = max(0, 128 * (jc - 4 * it))
                    nc.tensor.matmul(
                        ctx_ps[0:65, c0:IS],
                        v4[:, jc, h, :],
                        exps[:, qi, c0:IS],
                        start=(jc == 0), stop=(jc == njc - 1))
            return ctx_ps

        def recip_l(ctx_ps, h, it):
            # 1/l of row 64, written to partition 0. The custom-DVE
            # reciprocal needs an SBUF input (bitwise seed misreads PSUM),
            # so stage the l row through SBUF first.
            ls = rope_tmp.tile([1, IS], F32, tag="ls")
            nc.vector.tensor_copy(ls, ctx_ps[64:65, :])
            nc.vector.reciprocal_approx_fast(rl[0:1, h, it % 2, :], ls)

        def norm_and_outproj(ctx_tiles, it):
            ctxs_pair = []
            for pair in range(2):
                ctxs = ctxp.tile([128, IS], BF16, tag="c", name="ctxs")
                for half in range(2):
                    bcs = bcp.tile([64, IS], F32, tag="bc", name="bcs")
                    nc.gpsimd.partition_broadcast(
                        bcs, rl[0:1, 2 * pair + half, it % 2, :])
                    nc.vector.tensor_mul(
                        ctxs[64 * half:64 * half + 64, :],
                        ctx_tiles[2 * pair + half][0:64, :], bcs)
                ctxs_pair.append(ctxs)
            for ib in range(4):
                ot = outp.tile([128, 1024], BF16, tag="o", name="ot")
                for nt in range(2):
                    ops = ps_a.tile([128, IS], F32, tag="a", name="ops")
                    for pair in range(2):
                        nc.tensor.matmul(
                            ops,
                            ctxs_pair[pair][:, 128 * ib:128 * (ib + 1)],
                            wo_sb[:, pair, nt * IS:(nt + 1) * IS],
                            start=(pair == 0), stop=(pair == 1))
                    nc.vector.tensor_copy(ot[:, nt * IS:(nt + 1) * IS], ops)
                nc.sync.dma_start(
                    out=out[it * IS + 128 * ib: it * IS + 128 * (ib + 1), :],
                    in_=ot)

        # Issue order: norm(it-1) goes after head0(it)'s matmuls but BEFORE
        # recip(h0, it) so the DVE FIFO stays responsive.
        pending = None
        for it in range(NIS):
            ctx_tiles = {}
            ctx_tiles[0] = attention_head(0, 0, it)
            if pending is not None:
                norm_and_outproj(*pending)
            recip_l(ctx_tiles[0], 0, it)
            for h, (pair, half) in enumerate([(0, 1), (1, 0), (1, 1)], start=1):
                ctx_tiles[h] = attention_head(pair, half, it)
                recip_l(ctx_tiles[h], h, it)
            pending = (ctx_tiles, it)
        norm_and_outproj(*pending)

    nc.compile()
    return nc


def _host_tables():
    inv_freq = 1.0 / (THETA ** (np.arange(0, HD, 2, dtype=np.float64) / HD))
    pos = np.arange(S, dtype=np.float64)
    ang = pos[None, :] * inv_freq[:, None]          # [32, S]
    cos32 = np.cos(ang).astype(np.float32)
    sin32 = np.sin(ang).astype(np.float32)
    cdup = np.concatenate([cos32, cos32, cos32, cos32], axis=0)  # [128, S]
    s_signed = np.concatenate([-sin32, sin32, -sin32, sin32], axis=0)
    p64 = np.zeros((128, 128), dtype=np.float32)
    for m in range(128):
        blk = m - (m % 64)
        d = m % 64
        p64[blk + ((d + 32) % 64), m] = 1.0
    ident = np.eye(128, dtype=np.float32)
    # tneg[j, c] = NEG where j > c (strict lower in (j, col) == masked j > i)
    tneg = np.where(np.arange(128)[:, None] > np.arange(128)[None, :],
                    NEG, 0.0).astype(np.float32)
    return cdup, s_signed, p64, ident, tneg


_NC_CACHE = {}


def make_in_maps(x, W_q, W_k, W_v, W_o):
    cdup, sdup, p64, ident, tneg = _host_tables()
    ones = np.ones((128, 65), dtype=np.float32)

    def b16(a):
        return np.ascontiguousarray(a.astype(BFNP))

    def wshuf(w):  # [1024, 256] -> [128, 8, 256]
        return b16(w.reshape(8, 128, 256).transpose(1, 0, 2))

    cdup, sdup, p64, ident, tneg, ones = (b16(a) for a in
                                          (cdup, sdup, p64, ident, tneg, ones))
    in_maps = []
    for c in range(NCORES):
        b, g = divmod(c, 4)
        cols = slice(256 * g, 256 * (g + 1))
        # xtr[it, p, ch, i] = x[b][512it+i, 128ch+p]
        xtr = b16(x[b].reshape(NIS, IS, 8, 128).transpose(0, 3, 2, 1))
        in_maps.append({
            "xT": xtr,
            "wq": wshuf(W_q[:, cols]),
            "wk": wshuf(W_k[:, cols]),
            "wv": wshuf(W_v[:, cols]),
            "wo": b16(W_o[cols, :].reshape(2, 128, 1024).transpose(1, 0, 2)),
            "cdup": cdup, "sdup": sdup, "p64": p64, "ident": ident,
            "tneg": tneg, "onesc": ones,
        })
    return in_maps


def kernel(x, W_q, W_k, W_v, W_o):
    x = np.ascontiguousarray(x, dtype=np.float32)
    W_q = np.ascontiguousarray(W_q, dtype=np.float32)
    W_k = np.ascontiguousarray(W_k, dtype=np.float32)
    W_v = np.ascontiguousarray(W_v, dtype=np.float32)
    W_o = np.ascontiguousarray(W_o, dtype=np.float32)

    if "nc" not in _NC_CACHE:
        _NC_CACHE["nc"] = build_kernel()
    nc = _NC_CACHE["nc"]

    in_maps = make_in_maps(x, W_q, W_k, W_v, W_o)
    res = run_bass_kernel_spmd(nc, in_maps, list(range(NCORES)))
    outs = [res.results[c]["out"].astype(np.float32) for c in range(NCORES)]
    full = np.empty((B, S, D_OUT), dtype=np.float32)
    for b in range(B):
        full[b] = outs[4 * b] + outs[4 * b + 1] + outs[4 * b + 2] + outs[4 * b + 3]
    return full


# revision 5
# speedup vs baseline: 1.3769x; 1.0428x over previous
"""Multi-head attention (RoPE, causal) Bass kernel for 8 TRN2 NeuronCores.

Sharding: 2-way batch x 4-way heads (4 heads per core).

Schedule (v3): the kernel is software-pipelined at the instruction level so
the PE never idles (idling >3.4us re-throttles the PE clock to 1.2 GHz):
  - prologue: QKV+RoPE projection of i-slice 0
  - stage it: attention over slice it; projection MMs of slice it+1 and
    out-projection MMs of slice it-1 are interleaved into the attention
    stream as PE filler, paced by estimated duration
  - attention inner loop issues score-MMs of chunk-pair g+1 BEFORE the
    ctx-MMs of pair g, so the exp (ACT) latency of pair g is hidden
    behind PE work
Other mechanics:
  - bf16 data path (DMA halved, FWL weight loads); PSUM stays f32
  - causal mask folded into score PSUM via identity-matmul bias (-30k
    upper triangle): no DVE op between exp and ctx matmul
  - softmax 1/l via reciprocal_approx_fast off an SBUF-staged l row
  - softmax denominator comes free from a ones-column appended to V
  - norm of head h deferred to head h+1; out-proj of slice it deferred
    into stage it+1 as filler
  - input DMA dispatches spread across engine queues (parallel dispatch)
"""
import numpy as np
from contextlib import ExitStack

import ml_dtypes
import concourse.bass as bass
import concourse.tile as tile
from concourse import bacc, mybir
from concourse.bass_utils import run_bass_kernel_spmd

D_IN = 1024
D_OUT = 1024
HD = 64                   # head dim
S = 2048                  # sequence length
B = 2
THETA = 10000.0
NCORES = 8
IS = 512                  # i-slice width
NIS = S // IS             # 4 i-slices
NJC = S // 128            # 16 j-chunks
NEG = -30000.0            # causal mask bias (exp(NEG/8) == 0 in f32)

F32 = mybir.dt.float32
BF16 = mybir.dt.bfloat16
BFNP = ml_dtypes.bfloat16


class Fill:
    """Filler queue: PE work items popped into attention group gaps,
    paced so that `done/total` tracks the attention progress fraction."""

    def __init__(self):
        self.items = []
        self.total = 1e-9
        self.done = 0.0

    def add(self, est, fn):
        self.items.append((est, fn))
        self.total += est

    def pop(self, frac):
        target = frac * self.total
        while self.items and self.done < target:
            est, fn = self.items.pop(0)
            fn()
            self.done += est

    def drain(self):
        while self.items:
            est, fn = self.items.pop(0)
            fn()
            self.done += est


def build_kernel():
    nc = bacc.Bacc("TRN2", target_bir_lowering=False, debug=False)

    # host pre-shuffled so every DMA is contiguous per partition:
    # xtr[it, p, c, i] = x[b]^T[128c+p, 512it+i]; w*r[p, c, n] = W[128c+p, n]
    xT = nc.dram_tensor("xT", [NIS, 128, 8, IS], BF16, kind="ExternalInput").ap()
    wq = nc.dram_tensor("wq", [128, 8, 256], BF16, kind="ExternalInput").ap()
    wk = nc.dram_tensor("wk", [128, 8, 256], BF16, kind="ExternalInput").ap()
    wv = nc.dram_tensor("wv", [128, 8, 256], BF16, kind="ExternalInput").ap()
    wo = nc.dram_tensor("wo", [128, 2, 1024], BF16, kind="ExternalInput").ap()
    cdup = nc.dram_tensor("cdup", [128, S], BF16, kind="ExternalInput").ap()
    sdup = nc.dram_tensor("sdup", [128, S], BF16, kind="ExternalInput").ap()
    p64 = nc.dram_tensor("p64", [128, 128], BF16, kind="ExternalInput").ap()
    ident = nc.dram_tensor("ident", [128, 128], BF16, kind="ExternalInput").ap()
    tneg = nc.dram_tensor("tneg", [128, 128], BF16, kind="ExternalInput").ap()
    onesc = nc.dram_tensor("onesc", [128, 65], BF16, kind="ExternalInput").ap()
    out = nc.dram_tensor("out", [S, D_OUT], BF16, kind="ExternalOutput").ap()

    with tile.TileContext(nc) as tc, ExitStack() as ctx:
        singles = ctx.enter_context(tc.tile_pool(name="singles", bufs=1))
        xpool = ctx.enter_context(tc.tile_pool(name="xpool", bufs=2))
        rope_tmp = ctx.enter_context(tc.tile_pool(name="rope_tmp", bufs=3))
        expp = ctx.enter_context(tc.tile_pool(name="expp", bufs=3))
        bcp = ctx.enter_context(tc.tile_pool(name="bcp", bufs=2))
        ctxp = ctx.enter_context(tc.tile_pool(name="ctxp", bufs=4))
        outp = ctx.enter_context(tc.tile_pool(name="outp", bufs=3))
        # PSUM: quad 2x2 banks + ctx 2x1 + shared proj/perm/vps/ops 2x1 = 8
        ps = ctx.enter_context(tc.tile_pool(name="ps", bufs=2, space="PSUM"))

        # ---- input DMAs: dispatch spread across idle engine queues ----
        wq_t = singles.tile([128, 8, 256], BF16, tag="wq", name="wq")
        xt0_t = xpool.tile([128, 8, IS], BF16, tag="xt", name="xt0")
        for c in range(8):
            nc.sync.dma_start(out=wq_t[:, c, :], in_=wq[:, c, :])
            nc.sync.dma_start(out=xt0_t[:, c, :], in_=xT[0, :, c, :])
        xts = {0: xt0_t}

        def xt_dma(it):
            t = xpool.tile([128, 8, IS], BF16, tag="xt", name=f"xt{it}")
            nc.sync.dma_start(out=t, in_=xT[it])
            xts[it] = t

        wk_t = singles.tile([128, 8, 256], BF16, tag="wk", name="wk")
        for s in range(4):
            nc.gpsimd.dma_start(out=wk_t[:, 2 * s:2 * s + 2, :],
                                in_=wk[:, 2 * s:2 * s + 2, :])
        c_sb = singles.tile([128, S], BF16, tag="cdup")
        nc.scalar.dma_start(out=c_sb, in_=cdup)
        s_sb = singles.tile([128, S], BF16, tag="sdup")
        nc.scalar.dma_start(out=s_sb, in_=sdup)
        p64_sb = singles.tile([128, 128], BF16, tag="p64")
        nc.gpsimd.dma_start(out=p64_sb, in_=p64)
        wv_t = singles.tile([128, 8, 256], BF16, tag="wv", name="wv")
        nc.gpsimd.dma_start(out=wv_t, in_=wv)
        onesc_sb = singles.tile([128, 65], BF16, tag="ones")
        nc.gpsimd.dma_start(out=onesc_sb, in_=onesc)
        ident_sb = singles.tile([128, 128], BF16, tag="ident")
        nc.gpsimd.dma_start(out=ident_sb, in_=ident)
        tneg_sb = singles.tile([128, 128], BF16, tag="tneg")
        nc.gpsimd.dma_start(out=tneg_sb, in_=tneg)
        xt_dma(1)
        wo_sb = singles.tile([128, 2, 1024], BF16, tag="wo")
        nc.scalar.dma_start(out=wo_sb, in_=wo)
        w_sb = {"wq": wq_t, "wk": wk_t, "wv": wv_t}

        # persistent SBUF state
        qt = [singles.tile([128, S], BF16, tag=f"qt{p}", name=f"qt{p}")
              for p in range(2)]
        kt = [singles.tile([128, S], BF16, tag=f"kt{p}", name=f"kt{p}")
              for p in range(2)]
        v4 = singles.tile([128, NJC, 4, 65], BF16, tag="v4")
        v4_ones = bass.AP(tensor=v4.tensor, offset=64,
                          ap=[[NJC * 4 * 65, 128], [65, NJC * 4]])
        nc.vector.tensor_copy(v4_ones, onesc_sb[:, 0:64])
        # 1/l rows at partition 0: [head, it parity, i]
        rl = singles.tile([128, 4, 2, IS], F32, tag="rl")

        # ---------------- projection work items for slice jt --------------
        def proj_items(jt, fill):
            """Queue Q/K/V projection + RoPE of slice jt as filler items."""
            st = {}

            def qk_mm(tname, wt, p, c):
                def fn():
                    if c == 0:
                        st[(tname, p)] = ps.tile(
                            [128, IS], F32, tag="pp", name=f"proj_{tname}{p}")
                    proj = st[(tname, p)]
                    nc.tensor.matmul(proj, wt[:, c, 128 * p:128 * (p + 1)],
                                     xts[jt][:, c, :],
                                     start=(c == 0), stop=(c == 7))
                    if c == 7:
                        raw = rope_tmp.tile([128, IS], BF16, tag="raw")
                        nc.scalar.copy(raw, proj)
                        t1 = rope_tmp.tile([128, IS], BF16, tag="t1")
                        nc.vector.tensor_mul(
                            t1, proj, c_sb[:, jt * IS:(jt + 1) * IS])
                        st[(tname, p, "raw")] = raw
                        st[(tname, p, "t1")] = t1
                return (213, fn)

            def perm_item(tname, dests, p):
                def fn():
                    perm = ps.tile([128, IS], F32, tag="pp", name="perm")
                    nc.tensor.matmul(perm, p64_sb, st[(tname, p, "raw")],
                                     start=True, stop=True)
                    t2 = rope_tmp.tile([128, IS], BF16, tag="t2")
                    nc.vector.tensor_mul(
                        t2, perm, s_sb[:, jt * IS:(jt + 1) * IS])
                    nc.vector.tensor_add(
                        dests[p][:, jt * IS:(jt + 1) * IS],
                        st[(tname, p, "t1")], t2)
                return (230, fn)

            def v_mm(half, js, c):
                def fn():
                    if js == 0 and c == 0:
                        st[("v", half)] = ps.tile(
                            [128, 2, 256], F32, tag="pp", name=f"vps{half}")
                    vps = st[("v", half)]
                    col = 128 * (half * 2 + js)
                    nc.tensor.matmul(vps[:, js, :],
                                     xts[jt][:, c, col:col + 128],
                                     w_sb["wv"][:, c, :],
                                     start=(c == 0), stop=(c == 7))
                    if c == 7:
                        jtile = jt * 4 + half * 2 + js
                        nc.vector.tensor_copy(
                            v4[:, jtile, :, 0:64],
                            vps[:, js, :].rearrange("p (h d) -> p h d", h=4))
                return (120, fn)

            for c in range(8):
                fill.add(*qk_mm("q", w_sb["wq"], 0, c))
            for c in range(8):
                fill.add(*qk_mm("q", w_sb["wq"], 1, c))
            fill.add(*perm_item("q", qt, 0))
            for c in range(8):
                fill.add(*qk_mm("k", w_sb["wk"], 0, c))
            fill.add(*perm_item("q", qt, 1))
            for c in range(8):
                fill.add(*qk_mm("k", w_sb["wk"], 1, c))
            fill.add(*perm_item("k", kt, 0))
            for js in range(2):
                for c in range(8):
                    fill.add(*v_mm(0, js, c))
            fill.add(*perm_item("k", kt, 1))
            for js in range(2):
                for c in range(8):
                    fill.add(*v_mm(1, js, c))

        # ---------------- out-projection items for slice it ---------------
        def outproj_items(ctxs_pair, it, fill):
            st = {}

            def op_item(ib, nt):
                def fn():
                    if nt == 0:
                        st[ib] = outp.tile([128, 1024], BF16, tag="o",
                                           name="ot")
                    ot = st[ib]
                    ops = ps.tile([128, IS], F32, tag="pp", name="ops")
                    for pair in range(2):
                        nc.tensor.matmul(
                            ops,
                            ctxs_pair[pair][:, 128 * ib:128 * (ib + 1)],
                            wo_sb[:, pair, nt * IS:(nt + 1) * IS],
                            start=(pair == 0), stop=(pair == 1))
                    nc.vector.tensor_copy(ot[:, nt * IS:(nt + 1) * IS], ops)
                    if nt == 1:
                        nc.sync.dma_start(
                            out=out[it * IS + 128 * ib:
                                    it * IS + 128 * (ib + 1), :],
                            in_=ot)
                return (550, fn)

            for ib in range(4):
                for nt in range(2):
                    fill.add(*op_item(ib, nt))

        # ---------------- attention -----------------
        def recip_l(ctx_ps, h, it):
            # custom-DVE reciprocal needs an SBUF input (bitwise seed
            # misreads PSUM): stage the l row through SBUF first
            ls = rope_tmp.tile([1, IS], F32, tag="ls")
            nc.vector.tensor_copy(ls, ctx_ps[64:65, :])
            nc.vector.reciprocal_approx_fast(rl[0:1, h, it % 2, :], ls)

        def norm_head(ctx_tiles, ctxs_pair, h, it):
            """Scale head h's ctx rows by 1/l into the bf16 ctxs tile."""
            pair, half = divmod(h, 2)
            if half == 0:
                ctxs_pair[pair] = ctxp.tile([128, IS], BF16, tag="c",
                                            name="ctxs")
            bcs = bcp.tile([64, IS], F32, tag="bc", name="bcs")
            nc.gpsimd.partition_broadcast(bcs, rl[0:1, h, it % 2, :])
            nc.vector.tensor_mul(
                ctxs_pair[pair][64 * half:64 * half + 64, :],
                ctx_tiles[h][0:64, :], bcs)

        def attention_head(pair, half, it, fill, fbase, fstep):
            h = 2 * pair + half
            hb = 64 * half
            qs = qt[pair][hb:hb + 64, :]
            ks = kt[pair][hb:hb + 64, :]
            njc = 4 * it + 4
            ctx_ps = ps.tile([128, IS], F32, tag="ctx", name=f"ctx{h}")
            pend = None
            for gi, q0 in enumerate(range(0, njc, 2)):
                quad = ps.tile([128, 2, IS], F32, tag="quad", name="quad")
                exps = expp.tile([128, 2, IS], BF16, tag="e", name="exps")
                cmin = max(0, 128 * (q0 - 4 * it))
                recs = []
                for qi in range(2):
                    jc = q0 + qi
                    c0 = max(0, 128 * (jc - 4 * it))
                    diag = jc >= 4 * it
                    nc.tensor.matmul(
                        quad[:, qi, c0:IS],
                        ks[:, 128 * jc:128 * (jc + 1)],
                        qs[:, it * IS + c0:(it + 1) * IS],
                        start=True, stop=not diag)
                    if diag:
                        nc.tensor.matmul(
                            quad[:, qi, c0:c0 + 128],
                            ident_sb, tneg_sb,
                            start=False, stop=True)
                    recs.append((qi, jc, c0))
                nc.scalar.activation(
                    exps[:, 0:2, cmin:IS], quad[:, 0:2, cmin:IS],
                    mybir.ActivationFunctionType.Exp, scale=0.125)
                if pend is not None:
                    for qi, jc, c0 in pend[1]:
                        nc.tensor.matmul(
                            ctx_ps[0:65, c0:IS],
                            v4[:, jc, h, :],
                            pend[0][:, qi, c0:IS],
                            start=(jc == 0), stop=(jc == njc - 1))
                pend = (exps, recs)
                fill.pop(fbase + (gi + 1) * fstep)
            for qi, jc, c0 in pend[1]:
                nc.tensor.matmul(
                    ctx_ps[0:65, c0:IS],
                    v4[:, jc, h, :],
                    pend[0][:, qi, c0:IS],
                    start=(jc == 0), stop=(jc == njc - 1))
            recip_l(ctx_ps, h, it)
            return ctx_ps

        # ================= prologue: project slice 0 =================
        pro = Fill()
        proj_items(0, pro)
        pro.drain()

        # ================= stages =================
        HEADS = [(0, 0), (0, 1), (1, 0), (1, 1)]
        prev = None            # (ctx_tiles, ctxs_pair, it) of stage it-1
        for it in range(NIS):
            if it + 2 < NIS:
                xt_dma(it + 2)
            fill = Fill()
            if it + 1 < NIS:
                proj_items(it + 1, fill)
            groups = 2 * it + 2
            ctx_tiles = {}
            ctxs_pair = [None, None]
            for h, (pair, half) in enumerate(HEADS):
                # deferred norms: head h-1 of this stage, or h3 of previous
                if h == 0 and prev is not None:
                    norm_head(prev[0], prev[1], 3, prev[2])
                    outproj_items(prev[1], prev[2], fill)
                if h > 0:
                    norm_head(ctx_tiles, ctxs_pair, h - 1, it)
                fbase = h / 4.0
                fstep = 1.0 / (4.0 * groups)
                ctx_tiles[h] = attention_head(pair, half, it, fill,
                                              fbase, fstep)
            fill.drain()
            prev = (ctx_tiles, ctxs_pair, it)

        # ================= epilogue =================
        norm_head(prev[0], prev[1], 3, prev[2])
        epi = Fill()
        outproj_items(prev[1], prev[2], epi)
        epi.drain()

    nc.compile()
    return nc


def _host_tables():
    inv_freq = 1.0 / (THETA ** (np.arange(0, HD, 2, dtype=np.float64) / HD))
    pos = np.arange(S, dtype=np.float64)
    ang = pos[None, :] * inv_freq[:, None]          # [32, S]
    cos32 = np.cos(ang).astype(np.float32)
    sin32 = np.sin(ang).astype(np.float32)
    cdup = np.concatenate([cos32, cos32, cos32, cos32], axis=0)  # [128, S]
    s_signed = np.concatenate([-sin32, sin32, -sin32, sin32], axis=0)
    p64 = np.zeros((128, 128), dtype=np.float32)
    for m in range(128):
        blk = m - (m % 64)
        d = m % 64
        p64[blk + ((d + 32) % 64), m] = 1.0
    ident = np.eye(128, dtype=np.float32)
    # tneg[j, c] = NEG where j > c (strictly below the block diagonal)
    tneg = np.where(np.arange(128)[:, None] > np.arange(128)[None, :],
                    NEG, 0.0).astype(np.float32)
    return cdup, s_signed, p64, ident, tneg


_NC_CACHE = {}


def make_in_maps(x, W_q, W_k, W_v, W_o):
    cdup, sdup, p64, ident, tneg = _host_tables()
    ones = np.ones((128, 65), dtype=np.float32)

    def b16(a):
        return np.ascontiguousarray(a.astype(BFNP))

    def wshuf(w):  # [1024, 256] -> [128, 8, 256]
        return b16(w.reshape(8, 128, 256).transpose(1, 0, 2))

    cdup, sdup, p64, ident, tneg, ones = (b16(a) for a in
                                          (cdup, sdup, p64, ident, tneg, ones))
    in_maps = []
    for c in range(NCORES):
        b, g = divmod(c, 4)
        cols = slice(256 * g, 256 * (g + 1))
        # xtr[it, p, ch, i] = x[b][512it+i, 128ch+p]
        xtr = b16(x[b].reshape(NIS, IS, 8, 128).transpose(0, 3, 2, 1))
        in_maps.append({
            "xT": xtr,
            "wq": wshuf(W_q[:, cols]),
            "wk": wshuf(W_k[:, cols]),
            "wv": wshuf(W_v[:, cols]),
            "wo": b16(W_o[cols, :].reshape(2, 128, 1024).transpose(1, 0, 2)),
            "cdup": cdup, "sdup": sdup, "p64": p64, "ident": ident,
            "tneg": tneg, "onesc": ones,
        })
    return in_maps


def kernel(x, W_q, W_k, W_v, W_o):
    x = np.ascontiguousarray(x, dtype=np.float32)
    W_q = np.ascontiguousarray(W_q, dtype=np.float32)
    W_k = np.ascontiguousarray(W_k, dtype=np.float32)
    W_v = np.ascontiguousarray(W_v, dtype=np.float32)
    W_o = np.ascontiguousarray(W_o, dtype=np.float32)

    if "nc" not in _NC_CACHE:
        _NC_CACHE["nc"] = build_kernel()
    nc = _NC_CACHE["nc"]

    in_maps = make_in_maps(x, W_q, W_k, W_v, W_o)
    res = run_bass_kernel_spmd(nc, in_maps, list(range(NCORES)))
    outs = [res.results[c]["out"].astype(np.float32) for c in range(NCORES)]
    full = np.empty((B, S, D_OUT), dtype=np.float32)
    for b in range(B):
        full[b] = outs[4 * b] + outs[4 * b + 1] + outs[4 * b + 2] + outs[4 * b + 3]
    return full


# revision 7
# speedup vs baseline: 1.4732x; 1.0699x over previous
"""Multi-head attention (RoPE, causal) Bass kernel for 8 TRN2 NeuronCores.

Sharding: 2-way batch x 4-way heads (4 heads per core).

Schedule (v4): software-pipelined at the instruction level so the PE never
idles (idling >3.4us re-throttles the PE clock to 1.2 GHz):
  - prologue: projection of i-slice 0, pair-0 first; pair-1 projections are
    queued as stage-0 filler
  - stage it: attention over slice it; projection MMs of slice it+1 and
    out-projection MMs of slice it-1 interleave into the attention stream
    as PE filler, paced by estimated duration
  - attention issues score-MMs of chunk-pair g+1 BEFORE ctx-MMs of pair g
    so the exp (ACT) latency of pair g hides behind PE work
Mechanics:
  - bf16 data path (DMA halved, FWL weight loads); PSUM stays f32
  - causal mask folded into score PSUM via one identity-matmul bias per
    diagonal chunk pair (-30k upper triangle, skewed output AP)
  - softmax 1/l via reciprocal_approx_fast off an SBUF-staged l row;
    denominator comes free from a ones-column appended to V
  - norm of head h deferred to head h+1; out-proj of slice it deferred
    into stage it+1 as filler
  - input DMAs dispatched across engine queues in need-order; epilogue
    output DMAs fan out over four queues
"""
import numpy as np
from contextlib import ExitStack

import ml_dtypes
import concourse.bass as bass
import concourse.tile as tile
from concourse import bacc, mybir
from concourse.bass_utils import run_bass_kernel_spmd

D_IN = 1024
D_OUT = 1024
HD = 64                   # head dim
S = 2048                  # sequence length
B = 2
THETA = 10000.0
NCORES = 8
IS = 512                  # i-slice width
NIS = S // IS             # 4 i-slices
NJC = S // 128            # 16 j-chunks
NEG = -30000.0            # causal mask bias (exp(NEG/8) == 0 in f32)

F32 = mybir.dt.float32
BF16 = mybir.dt.bfloat16
BFNP = ml_dtypes.bfloat16


class Fill:
    """Filler queue: PE work items popped into attention group gaps,
    paced so that `done/total` tracks the attention progress fraction."""

    def __init__(self):
        self.items = []
        self.total = 1e-9
        self.done = 0.0
        self.markers = {}
        self._next_marker = 0

    def add(self, est, fn):
        self.items.append((est, fn))
        self.total += est

    def add_marker(self):
        mid = self._next_marker
        self._next_marker += 1
        self.markers[mid] = len(self.items)
        return mid

    def pop(self, frac):
        target = frac * self.total
        while self.items and self.done < target:
            self._pop_one()

    def pop_marker(self, mid):
        while self.markers.get(mid, 0) > 0 and self.items:
            self._pop_one()

    def _pop_one(self):
        est, fn = self.items.pop(0)
        fn()
        self.done += est
        for k in self.markers:
            if self.markers[k] > 0:
                self.markers[k] -= 1

    def drain(self):
        while self.items:
            self._pop_one()


def build_kernel():
    nc = bacc.Bacc("TRN2", target_bir_lowering=False, debug=False)

    # host pre-shuffled so every DMA is contiguous per partition:
    # xtr[it, p, c, i] = x[b]^T[128c+p, 512it+i]; w*r[p, c, n] = W[128c+p, n]
    xT = nc.dram_tensor("xT", [NIS, 128, 8, IS], BF16, kind="ExternalInput").ap()
    wq = nc.dram_tensor("wq", [128, 8, 256], BF16, kind="ExternalInput").ap()
    wk = nc.dram_tensor("wk", [128, 8, 256], BF16, kind="ExternalInput").ap()
    wv = nc.dram_tensor("wv", [128, 8, 256], BF16, kind="ExternalInput").ap()
    wo = nc.dram_tensor("wo", [128, 2, 1024], BF16, kind="ExternalInput").ap()
    cdup = nc.dram_tensor("cdup", [128, S], BF16, kind="ExternalInput").ap()
    sdup = nc.dram_tensor("sdup", [128, S], BF16, kind="ExternalInput").ap()
    p64 = nc.dram_tensor("p64", [128, 128], BF16, kind="ExternalInput").ap()
    ident = nc.dram_tensor("ident", [128, 128], BF16, kind="ExternalInput").ap()
    # [tri | tri]: one bias matmul covers both chunks of a diagonal pair
    tneg2 = nc.dram_tensor("tneg2", [128, 256], BF16, kind="ExternalInput").ap()
    onesc = nc.dram_tensor("onesc", [128, 65], BF16, kind="ExternalInput").ap()
    out = nc.dram_tensor("out", [S, D_OUT], BF16, kind="ExternalOutput").ap()

    with tile.TileContext(nc) as tc, ExitStack() as ctx:
        singles = ctx.enter_context(tc.tile_pool(name="singles", bufs=1))
        xpool = ctx.enter_context(tc.tile_pool(name="xpool", bufs=2))
        rope_tmp = ctx.enter_context(tc.tile_pool(name="rope_tmp", bufs=3))
        expp = ctx.enter_context(tc.tile_pool(name="expp", bufs=4))
        bcp = ctx.enter_context(tc.tile_pool(name="bcp", bufs=2))
        ctxp = ctx.enter_context(tc.tile_pool(name="ctxp", bufs=4))
        outp = ctx.enter_context(tc.tile_pool(name="outp", bufs=3))
        # PSUM: quad 2x2 banks + ctx 2x1 + shared proj/perm/vps/ops 2x1 = 8
        ps = ctx.enter_context(tc.tile_pool(name="ps", bufs=2, space="PSUM"))

        # ---- input DMAs: need-ordered, dispatch spread across engines ----
        engs = [nc.sync, nc.scalar, nc.gpsimd]
        wq_t = singles.tile([128, 8, 256], BF16, tag="wq", name="wq")
        xt0_t = xpool.tile([128, 8, IS], BF16, tag="xt", name="xt0")
        for c in range(8):
            e = engs[c % 3]
            e.dma_start(out=wq_t[:, c, :], in_=wq[:, c, :])
            e.dma_start(out=xt0_t[:, c, :], in_=xT[0, :, c, :])
        xts = {0: xt0_t}
        # mid-prologue needs: wk (k-proj), cdup (t1), p64 (perm)
        wk_t = singles.tile([128, 8, 256], BF16, tag="wk", name="wk")
        nc.gpsimd.dma_start(out=wk_t[:, 0:4, :], in_=wk[:, 0:4, :])
        c_sb = singles.tile([128, S], BF16, tag="cdup")
        nc.scalar.dma_start(out=c_sb, in_=cdup)
        p64_sb = singles.tile([128, 128], BF16, tag="p64")
        nc.gpsimd.dma_start(out=p64_sb, in_=p64)
        s_sb = singles.tile([128, S], BF16, tag="sdup")
        nc.sync.dma_start(out=s_sb, in_=sdup)
        nc.gpsimd.dma_start(out=wk_t[:, 4:8, :], in_=wk[:, 4:8, :])
        wv_t = singles.tile([128, 8, 256], BF16, tag="wv", name="wv")
        nc.scalar.dma_start(out=wv_t, in_=wv)

        def xt_dma(it):
            t = xpool.tile([128, 8, IS], BF16, tag="xt", name=f"xt{it}")
            nc.sync.dma_start(out=t, in_=xT[it])
            xts[it] = t

        xt_dma(1)
        onesc_sb = singles.tile([128, 65], BF16, tag="ones")
        nc.scalar.dma_start(out=onesc_sb, in_=onesc)
        ident_sb = singles.tile([128, 128], BF16, tag="ident")
        nc.gpsimd.dma_start(out=ident_sb, in_=ident)
        tneg2_sb = singles.tile([128, 256], BF16, tag="tneg2")
        nc.gpsimd.dma_start(out=tneg2_sb, in_=tneg2)
        wo_sb = singles.tile([128, 2, 1024], BF16, tag="wo")
        nc.scalar.dma_start(out=wo_sb, in_=wo)
        w_sb = {"wq": wq_t, "wk": wk_t, "wv": wv_t}

        # persistent SBUF state
        qt = [singles.tile([128, S], BF16, tag=f"qt{p}", name=f"qt{p}")
              for p in range(2)]
        kt = [singles.tile([128, S], BF16, tag=f"kt{p}", name=f"kt{p}")
              for p in range(2)]
        v4 = singles.tile([128, NJC, 4, 65], BF16, tag="v4")
        v4_ones = bass.AP(tensor=v4.tensor, offset=64,
                          ap=[[NJC * 4 * 65, 128], [65, NJC * 4]])
        nc.vector.tensor_copy(v4_ones, onesc_sb[:, 0:64])
        # 1/l rows at partition 0: [head, it parity, i]
        rl = singles.tile([128, 4, 2, IS], F32, tag="rl")

        # ---------------- projection work-item builders --------------
        def qk_items(jt, tname, p, st, fill):
            wt = w_sb["w" + tname]
            dests = qt if tname == "q" else kt

            def mk(c):
                def fn():
                    if c == 0:
                        st[(tname, p)] = ps.tile(
                            [128, IS], F32, tag="pp", name=f"proj_{tname}{p}")
                    proj = st[(tname, p)]
                    nc.tensor.matmul(proj, wt[:, c, 128 * p:128 * (p + 1)],
                                     xts[jt][:, c, :],
                                     start=(c == 0), stop=(c == 7))
                    if c == 7:
                        raw = rope_tmp.tile([128, IS], BF16, tag="raw")
                        nc.vector.tensor_copy(raw, proj)
                        t1 = rope_tmp.tile([128, IS], BF16, tag="t1")
                        nc.vector.tensor_mul(
                            t1, proj, c_sb[:, jt * IS:(jt + 1) * IS])
                        st[(tname, p, "raw")] = raw
                        st[(tname, p, "t1")] = t1
                return (213, fn)

            for c in range(8):
                fill.add(*mk(c))

        def perm_item(jt, tname, p, st, fill):
            dests = qt if tname == "q" else kt

            def fn():
                perm = ps.tile([128, IS], F32, tag="pp", name="perm")
                nc.tensor.matmul(perm, p64_sb, st[(tname, p, "raw")],
                                 start=True, stop=True)
                t2 = rope_tmp.tile([128, IS], BF16, tag="t2")
                nc.vector.tensor_mul(t2, perm, s_sb[:, jt * IS:(jt + 1) * IS])
                nc.vector.tensor_add(
                    dests[p][:, jt * IS:(jt + 1) * IS],
                    st[(tname, p, "t1")], t2)
            fill.add(230, fn)

        def v_items(jt, half, st, fill):
            def mk(js, c):
                def fn():
                    if js == 0 and c == 0:
                        st[("v", half)] = ps.tile(
                            [128, 2, 256], F32, tag="pp", name=f"vps{half}")
                    vps = st[("v", half)]
                    col = 128 * (half * 2 + js)
                    nc.tensor.matmul(vps[:, js, :],
                                     xts[jt][:, c, col:col + 128],
                                     w_sb["wv"][:, c, :],
                                     start=(c == 0), stop=(c == 7))
                    if c == 7:
                        jtile = jt * 4 + half * 2 + js
                        nc.vector.tensor_copy(
                            v4[:, jtile, :, 0:64],
                            vps[:, js, :].rearrange("p (h d) -> p h d", h=4))
                return (120, fn)

            for js in range(2):
                for c in range(8):
                    fill.add(*mk(js, c))

        def proj_items(jt, fill):
            st = {}
            qk_items(jt, "q", 0, st, fill)
            qk_items(jt, "q", 1, st, fill)
            perm_item(jt, "q", 0, st, fill)
            qk_items(jt, "k", 0, st, fill)
            perm_item(jt, "q", 1, st, fill)
            qk_items(jt, "k", 1, st, fill)
            perm_item(jt, "k", 0, st, fill)
            v_items(jt, 0, st, fill)
            perm_item(jt, "k", 1, st, fill)
            v_items(jt, 1, st, fill)

        # ---------------- out-projection items for slice it ---------------
        def outproj_items(ctxs_pair, it, fill, epilogue=False):
            st = {}

            def op_item(ib, nt):
                def fn():
                    if nt == 0:
                        st[ib] = outp.tile([128, 1024], BF16, tag="o",
                                           name="ot")
                    ot = st[ib]
                    # epilogue: alternate PSUM tags for a deeper pipeline
                    tag = ("ctx" if (epilogue and ib % 2) else "pp")
                    ops = ps.tile([128, IS], F32, tag=tag, name="ops")
                    for pair in range(2):
                        nc.tensor.matmul(
                            ops,
                            ctxs_pair[pair][:, 128 * ib:128 * (ib + 1)],
                            wo_sb[:, pair, nt * IS:(nt + 1) * IS],
                            start=(pair == 0), stop=(pair == 1))
                    nc.vector.tensor_copy(ot[:, nt * IS:(nt + 1) * IS], ops)
                    if nt == 1:
                        eng = engs[ib % 3] if epilogue else nc.sync
                        eng.dma_start(
                            out=out[it * IS + 128 * ib:
                                    it * IS + 128 * (ib + 1), :],
                            in_=ot)
                return (550, fn)

            for ib in range(4):
                for nt in range(2):
                    fill.add(*op_item(ib, nt))

        # ---------------- attention -----------------
        def recip_l(ctx_ps, h, it):
            # custom-DVE reciprocal needs an SBUF input (bitwise seed
            # misreads PSUM): stage the l row through SBUF first
            ls = rope_tmp.tile([1, IS], F32, tag="ls")
            nc.vector.tensor_copy(ls, ctx_ps[64:65, :])
            nc.vector.reciprocal_approx_fast(rl[0:1, h, it % 2, :], ls)

        def norm_head(ctx_tiles, ctxs_pair, h, it):
            """Scale head h's ctx rows by 1/l into the bf16 ctxs tile."""
            pair, half = divmod(h, 2)
            if half == 0:
                ctxs_pair[pair] = ctxp.tile([128, IS], BF16, tag="c",
                                            name="ctxs")
            bcs = bcp.tile([64, IS], F32, tag="bc", name="bcs")
            nc.gpsimd.partition_broadcast(bcs, rl[0:1, h, it % 2, :])
            nc.vector.tensor_mul(
                ctxs_pair[pair][64 * half:64 * half + 64, :],
                ctx_tiles[h][0:64, :], bcs)

        def attention_head(pair, half, it, fill, fbase, fstep):
            h = 2 * pair + half
            hb = 64 * half
            qs = qt[pair][hb:hb + 64, :]
            ks = kt[pair][hb:hb + 64, :]
            njc = 4 * it + 4
            ctx_ps = ps.tile([128, IS], F32, tag="ctx", name=f"ctx{h}")
            pend = None
            for gi, q0 in enumerate(range(0, njc, 2)):
                quad = ps.tile([128, 2, IS], F32, tag="quad", name="quad")
                exps = expp.tile([128, 2, IS], BF16, tag="e", name="exps")
                cmin = max(0, 128 * (q0 - 4 * it))
                recs = []
                for qi in range(2):
                    jc = q0 + qi
                    c0 = max(0, 128 * (jc - 4 * it))
                    diag = jc >= 4 * it
                    nc.tensor.matmul(
                        quad[:, qi, c0:IS],
                        ks[:, 128 * jc:128 * (jc + 1)],
                        qs[:, it * IS + c0:(it + 1) * IS],
                        start=True, stop=not diag)
                    recs.append((qi, jc, c0))
                if q0 >= 4 * it:
                    # one skewed-AP bias matmul adds the -30k triangle into
                    # both diagonal chunks: cols [cmin:cmin+128] for qi=0,
                    # [cmin+128:cmin+256] for qi=1
                    skew = bass.AP(tensor=quad.tensor, offset=cmin,
                                   ap=[[2 * IS, 128], [IS + 128, 2], [1, 128]])
                    nc.tensor.matmul(skew, ident_sb, tneg2_sb,
                                     start=False, stop=True,
                                     skip_group_check=True)
                nc.scalar.activation(
                    exps[:, 0:2, cmin:IS], quad[:, 0:2, cmin:IS],
                    mybir.ActivationFunctionType.Exp, scale=0.125)
                if pend is not None:
                    for qi, jc, c0 in pend[1]:
                        nc.tensor.matmul(
                            ctx_ps[0:65, c0:IS],
                            v4[:, jc, h, :],
                            pend[0][:, qi, c0:IS],
                            start=(jc == 0), stop=(jc == njc - 1))
                pend = (exps, recs)
                fill.pop(fbase + (gi + 1) * fstep)
            for qi, jc, c0 in pend[1]:
                nc.tensor.matmul(
                    ctx_ps[0:65, c0:IS],
                    v4[:, jc, h, :],
                    pend[0][:, qi, c0:IS],
                    start=(jc == 0), stop=(jc == njc - 1))
            recip_l(ctx_ps, h, it)
            return ctx_ps

        # ========== prologue: slice-0 pair-0 projections + V ==========
        pro = Fill()
        st0 = {}
        qk_items(0, "q", 0, st0, pro)
        qk_items(0, "k", 0, st0, pro)
        perm_item(0, "q", 0, st0, pro)
        v_items(0, 0, st0, pro)
        perm_item(0, "k", 0, st0, pro)
        v_items(0, 1, st0, pro)
        pro.drain()

        # ================= stages =================
        HEADS = [(0, 0), (0, 1), (1, 0), (1, 1)]
        prev = None            # (ctx_tiles, ctxs_pair, it) of stage it-1
        for it in range(NIS):
            if it + 2 < NIS:
                xt_dma(it + 2)
            fill = Fill()
            pair1_marker = None
            if it == 0:
                # slice-0 pair-1 projections must land before heads 2-3
                qk_items(0, "q", 1, st0, fill)
                qk_items(0, "k", 1, st0, fill)
                perm_item(0, "q", 1, st0, fill)
                perm_item(0, "k", 1, st0, fill)
                pair1_marker = fill.add_marker()
            if it + 1 < NIS:
                proj_items(it + 1, fill)
            groups = 2 * it + 2
            ctx_tiles = {}
            ctxs_pair = [None, None]
            for h, (pair, half) in enumerate(HEADS):
                if h == 0 and prev is not None:
                    norm_head(prev[0], prev[1], 3, prev[2])
                    outproj_items(prev[1], prev[2], fill)
                if h > 0:
                    norm_head(ctx_tiles, ctxs_pair, h - 1, it)
                if h == 2 and pair1_marker is not None:
                    fill.pop_marker(pair1_marker)
                fbase = h / 4.0
                fstep = 1.0 / (4.0 * groups)
                ctx_tiles[h] = attention_head(pair, half, it, fill,
                                              fbase, fstep)
            fill.drain()
            prev = (ctx_tiles, ctxs_pair, it)

        # ================= epilogue =================
        norm_head(prev[0], prev[1], 3, prev[2])
        epi = Fill()
        outproj_items(prev[1], prev[2], epi, epilogue=True)
        epi.drain()

    nc.compile()
    return nc


def _host_tables():
    inv_freq = 1.0 / (THETA ** (np.arange(0, HD, 2, dtype=np.float64) / HD))
    pos = np.arange(S, dtype=np.float64)
    ang = pos[None, :] * inv_freq[:, None]          # [32, S]
    cos32 = np.cos(ang).astype(np.float32)
    sin32 = np.sin(ang).astype(np.float32)
    cdup = np.concatenate([cos32, cos32, cos32, cos32], axis=0)  # [128, S]
    s_signed = np.concatenate([-sin32, sin32, -sin32, sin32], axis=0)
    p64 = np.zeros((128, 128), dtype=np.float32)
    for m in range(128):
        blk = m - (m % 64)
        d = m % 64
        p64[blk + ((d + 32) % 64), m] = 1.0
    ident = np.eye(128, dtype=np.float32)
    # tneg[j, c] = NEG where j > c (strictly below the block diagonal)
    tneg = np.where(np.arange(128)[:, None] > np.arange(128)[None, :],
                    NEG, 0.0).astype(np.float32)
    tneg2 = np.concatenate([tneg, tneg], axis=1)
    return cdup, s_signed, p64, ident, tneg2


_NC_CACHE = {}


def make_in_maps(x, W_q, W_k, W_v, W_o):
    cdup, sdup, p64, ident, tneg2 = _host_tables()
    ones = np.ones((128, 65), dtype=np.float32)

    def b16(a):
        return np.ascontiguousarray(a.astype(BFNP))

    def wshuf(w):  # [1024, 256] -> [128, 8, 256]
        return b16(w.reshape(8, 128, 256).transpose(1, 0, 2))

    cdup, sdup, p64, ident, tneg2, ones = (b16(a) for a in
                                           (cdup, sdup, p64, ident, tneg2,
                                            ones))
    in_maps = []
    for c in range(NCORES):
        b, g = divmod(c, 4)
        cols = slice(256 * g, 256 * (g + 1))
        # xtr[it, p, ch, i] = x[b][512it+i, 128ch+p]
        xtr = b16(x[b].reshape(NIS, IS, 8, 128).transpose(0, 3, 2, 1))
        in_maps.append({
            "xT": xtr,
            "wq": wshuf(W_q[:, cols]),
            "wk": wshuf(W_k[:, cols]),
            "wv": wshuf(W_v[:, cols]),
            "wo": b16(W_o[cols, :].reshape(2, 128, 1024).transpose(1, 0, 2)),
            "cdup": cdup, "sdup": sdup, "p64": p64, "ident": ident,
            "tneg2": tneg2, "onesc": ones,
        })
    return in_maps


def kernel(x, W_q, W_k, W_v, W_o):
    x = np.ascontiguousarray(x, dtype=np.float32)
    W_q = np.ascontiguousarray(W_q, dtype=np.float32)
    W_k = np.ascontiguousarray(W_k, dtype=np.float32)
    W_v = np.ascontiguousarray(W_v, dtype=np.float32)
    W_o = np.ascontiguousarray(W_o, dtype=np.float32)

    if "nc" not in _NC_CACHE:
        _NC_CACHE["nc"] = build_kernel()
    nc = _NC_CACHE["nc"]

    in_maps = make_in_maps(x, W_q, W_k, W_v, W_o)
    res = run_bass_kernel_spmd(nc, in_maps, list(range(NCORES)))
    outs = [res.results[c]["out"].astype(np.float32) for c in range(NCORES)]
    full = np.empty((B, S, D_OUT), dtype=np.float32)
    for b in range(B):
        full[b] = outs[4 * b] + outs[4 * b + 1] + outs[4 * b + 2] + outs[4 * b + 3]
    return full
